# revision 1
# baseline (speedup 1.0000x reference)
"""Trainium2 Bass kernel for nn_LoLGNN (2-layer hetero GraphSAGE + pooling).

Sharding: graph-data parallel over 8 NeuronCores. Core c owns players
[25000c, 25000(c+1)) and graphs [2500c, 2500(c+1)). Edges are assigned to the
core owning their dst player and sorted by dst.

Device algorithm (per core), everything via one generic "aggregation machine":
  stream = (table[N, W] in DRAM, per-chunk idx[128], dstloc[128], invdeg[128])
  per chunk: indirect-gather V[128, W] = table[idx]; DVE builds the
  invdeg-scaled one-hot oh[e, d] = (dstloc[e]==d) * invdeg[e]; PE accumulates
  psum_sT[W, d] += V.T @ oh over a 128-dst window.
Phases:
  ENC: identity "graph" over own players; streams x_pad16/T12/T34/T5 -> p0
  HIST: streams xh_pad16 and T_h (combined emb table), persisted for L1+L2
  AllGather p0 -> p0_full; L1: streams p0_full (teammate, enemy) + hist +
  Wr term + bias -> relu -> p1; AllGather p1; L2 same -> p2 -> pooling -> out.
"""
import os
import sys

sys.path.insert(0, "/opt/trn_rl_repo")

import numpy as np

import concourse.bacc as bacc
import concourse.bass as bass
import concourse.tile as tile
import concourse.tile as tile_mod
from concourse import mybir
from concourse.bass_utils import run_bass_kernel_spmd
from bass_rust import ScopedClock, VectorClock

# ---------------------------------------------------------------- constants
N_PLAYER = 200000
N_HIST = 500000
N_GRAPH = 20000
H = 128
NC = 8
PC = N_PLAYER // NC      # players per core
GC = N_GRAPH // NC       # graphs per core
HC = N_HIST // NC
P = 128
F32 = mybir.dt.float32
I32 = mybir.dt.int32

LAST_EXEC_NS = [None]

# ------------------------------------------------- tail-drain walrus patch
_N_PROCS = 27


def _patched_drain_and_barrier(self, tick_clock, wait_clock):
    gc = tick_clock.global_clock
    nonzero = [p for p in range(_N_PROCS) if gc[p] > 0]
    if not nonzero:
        d = self.nc.sync.drain()
        wait_clock.add_sem_waits(d.ins, ScopedClock({None: gc.copy()}))
    for p in nonzero:
        vec = [0] * _N_PROCS
        vec[p] = gc[p]
        d = self.nc.sync.drain()
        wait_clock.add_sem_waits(d.ins, ScopedClock({None: VectorClock(vec)}))
    self.nc.all_engine_barrier()
    assert self.sems is not None
    popped = self.nc._tile_sem_poison_stack.pop()
    assert popped is self._sem_poison
    self.nc.clear_and_free_semaphores(list(self.sems.allocated().values()))
    self.nc.all_engine_barrier()


tile_mod.TileContext._drain_and_barrier = _patched_drain_and_barrier


# ------------------------------------------------------------- host helpers
def _ids(x, lo, hi):
    return np.clip(x.astype(np.int64), lo, hi)


def _sorted_stream(dst_local, idx_vals, deg, nwin, pad_idx=0):
    """Sort edges by local dst; chunk into 128-edge chunks; emit per-chunk
    jobs (window, dstloc row, invdeg row). Returns (idx[128, nchunk] i32,
    dstinv[128, 2*nchunk] f32, jobs list per chunk)."""
    order = np.argsort(dst_local, kind="stable")
    d = dst_local[order]
    v = idx_vals[order]
    e = len(d)
    nchunk = max(1, (e + P - 1) // P)
    epad = nchunk * P
    dp = np.full(epad, -1, np.int64)
    vp = np.full(epad, pad_idx, np.int64)
    dp[:e] = d
    vp[:e] = v
    inv = np.zeros(epad, np.float32)
    inv[:e] = 1.0 / np.maximum(deg[d], 1.0)
    idx = vp.reshape(nchunk, P).T.astype(np.int32).copy()      # [128, nchunk]
    jobs = []
    dstinv = np.zeros((P, 2 * nchunk), np.float32)
    for j in range(nchunk):
        dj = dp[j * P:(j + 1) * P]
        ij = inv[j * P:(j + 1) * P]
        wins = sorted(set(int(w) for w in dj[dj >= 0] // P))
        if not wins:
            wins = [0]
        cjobs = []
        for w in wins:
            loc = np.where((dj >= 0) & (dj // P == w), dj - w * P, -1.0)
            scl = np.where(loc >= 0, ij, 0.0)
            cjobs.append((w, loc.astype(np.float32), scl.astype(np.float32)))
        jobs.append(cjobs)
    # flatten job rows into dstinv columns; device reads per job
    flat = []
    for j, cjobs in enumerate(jobs):
        for (w, loc, scl) in cjobs:
            flat.append((j, w, loc, scl))
    njob = len(flat)
    dstinv = np.zeros((P, 2 * njob), np.float32)
    jmeta = []
    for k, (j, w, loc, scl) in enumerate(flat):
        dstinv[:, 2 * k] = loc
        dstinv[:, 2 * k + 1] = scl
        jmeta.append((j, w))
    return idx, dstinv, jmeta, nchunk, njob


def _pad_cols(a, cols):
    out = np.zeros((a.shape[0], cols), a.dtype)
    out[:, :a.shape[1]] = a
    return out


def _prep(inputs):
    """Host preprocessing -> per-core input maps + static shape config."""
    f32 = np.float32
    x_player = np.asarray(inputs["x_player"], f32)
    x_history = np.asarray(inputs["x_history"], f32)
    e_tm = np.asarray(inputs["edge_teammate"], np.int64)
    e_en = np.asarray(inputs["edge_enemy"], np.int64)
    e_h = np.asarray(inputs["edge_hist"], np.int64)
    emb_player = np.asarray(inputs["emb_player"], f32)
    emb_h0 = np.asarray(inputs["emb_h0"], f32)
    emb_h3 = np.asarray(inputs["emb_h3"], f32)
    Wp = np.asarray(inputs["Wp"], f32)
    bp = np.asarray(inputs["bp"], f32)
    Wh = np.asarray(inputs["Wh"], f32)
    bh = np.asarray(inputs["bh"], f32)
    sage_Wl = np.asarray(inputs["sage_Wl"], f32)
    sage_b = np.asarray(inputs["sage_b"], f32)
    sage_Wr = np.asarray(inputs["sage_Wr"], f32)
    Wc = np.asarray(inputs["Wc"], f32)
    bc = np.asarray(inputs["bc"], f32)

    nwin = (PC + P - 1) // P

    # padded feature tables (layout prep only)
    x_pad16 = np.zeros((N_PLAYER, 16), f32)
    x_pad16[:, :10] = x_player
    xh_pad16 = np.zeros((N_HIST, 16), f32)
    xh_pad16[:, :8] = x_history

    # weight-derived tables
    ids_p = _ids(x_player[:, 1:6], 0, 199)          # [N_PLAYER, 5]
    T12 = np.concatenate(
        [np.repeat(emb_player[0], 200, 0),
         np.tile(emb_player[1], (200, 1))], 1)       # [40000, 32]
    T34 = np.concatenate(
        [np.repeat(emb_player[2], 200, 0),
         np.tile(emb_player[3], (200, 1))], 1)
    T5 = emb_player[4]                               # [200, 16]
    id12 = ids_p[:, 0] * 200 + ids_p[:, 1]
    id34 = ids_p[:, 2] * 200 + ids_p[:, 3]
    id5 = ids_p[:, 4]

    idh0 = _ids(x_history[:, 0], 0, 1999)
    idh3 = _ids(x_history[:, 3], 0, 9)
    T_h = np.zeros((20000, 32), f32)                 # [id0*10+id3] -> emb0|emb3
    T_h[:, :16] = np.repeat(emb_h0, 10, 0)
    T_h[:, 16:20] = np.tile(emb_h3, (2000, 1))
    idh = idh0 * 10 + idh3

    # encoder weight slices (as W^T [W_s, 128])
    Wx_enc = np.zeros((16, H), f32)
    cont_cols = [0, 6, 7, 8, 9]
    for k, c in enumerate(cont_cols):
        Wx_enc[c] = Wp[:, k]
    W12T = Wp[:, 5:37].T.copy()
    W34T = Wp[:, 37:69].T.copy()
    W5T = Wp[:, 69:85].T.copy()

    # hist weight slices
    Wh_x = np.zeros((16, H), f32)
    hcols = [1, 2, 4, 5, 6, 7]
    for k, c in enumerate(hcols):
        Wh_x[c] = Wh[:, k]
    Wh_t = np.zeros((32, H), f32)
    Wh_t[:16] = Wh[:, 6:22].T
    Wh_t[16:20] = Wh[:, 22:26].T

    per_layer = []
    for l in range(2):
        Wl2 = sage_Wl[l, 2]
        per_layer.append(dict(
            WlT_tm=sage_Wl[l, 0].T.copy(),
            WlT_en=sage_Wl[l, 1].T.copy(),
            Ch_x=(Wl2 @ Wh_x.T).T.copy(),         # wait: hist term uses Wl2 @ enc
            Ch_t=(Wl2 @ Wh_t.T).T.copy(),
            WrT=(sage_Wr[l, 0] + sage_Wr[l, 1] + sage_Wr[l, 2]).T.copy(),
            bias=(sage_b[l].sum(0) + Wl2 @ bh).astype(f32),
        ))

    # per-core streams
    cores = []
    deg_store = []
    for c in range(NC):
        base = c * PC
        m_tm = (e_tm[1] >= base) & (e_tm[1] < base + PC)
        m_en = (e_en[1] >= base) & (e_en[1] < base + PC)
        m_h = (e_h[1] >= base) & (e_h[1] < base + PC)
        d_tm = e_tm[1][m_tm] - base
        s_tm = e_tm[0][m_tm]
        d_en = e_en[1][m_en] - base
        s_en = e_en[0][m_en]
        d_h = e_h[1][m_h] - base
        s_h = e_h[0][m_h]
        deg_tm = np.bincount(d_tm, minlength=PC).astype(f32)
        deg_en = np.bincount(d_en, minlength=PC).astype(f32)
        deg_h = np.bincount(d_h, minlength=PC).astype(f32)

        own = np.arange(base, base + PC)
        dloc_enc = np.arange(PC)
        ones = np.ones(PC, f32)

        streams = dict(
            enc_x=_sorted_stream(dloc_enc, own, ones, nwin),
            enc_12=_sorted_stream(dloc_enc, id12[own], ones, nwin),
            enc_34=_sorted_stream(dloc_enc, id34[own], ones, nwin),
            enc_5=_sorted_stream(dloc_enc, id5[own], ones, nwin),
            h_x=_sorted_stream(d_h, s_h, deg_h, nwin),
            h_t=_sorted_stream(d_h, idh[s_h], deg_h, nwin),
            tm=_sorted_stream(d_tm, s_tm, deg_tm, nwin),
            en=_sorted_stream(d_en, s_en, deg_en, nwin),
        )
        cores.append(streams)
        deg_store.append((deg_tm, deg_en, deg_h))

    # SPMD: shared chunk/job structure = union over cores
    names = list(cores[0].keys())
    shape_cfg = {}
    jmeta_shared = {}
    for n in names:
        nchunk = max(cores[c][n][3] for c in range(NC))
        union = set()
        for c in range(NC):
            union.update(cores[c][n][2])
        meta = sorted(union)
        jmeta_shared[n] = meta
        shape_cfg[n] = (nchunk, len(meta))

    in_maps = []
    for c in range(NC):
        m = {}
        for n in names:
            idx, dstinv, jmeta, nchunk, njob = cores[c][n]
            nck, njb = shape_cfg[n]
            idxp = _pad_cols(idx, nck)
            dv = np.zeros((P, 2 * njb), np.float32)
            # re-emit this core's jobs aligned to the SHARED job list:
            # shared job k = (chunk j, window w). If this core has a matching
            # job, use its rows; else all -1 (no-op).
            own_jobs = {}
            for k, (j, w) in enumerate(jmeta):
                own_jobs[(j, w)] = (dstinv[:, 2 * k], dstinv[:, 2 * k + 1])
            for k, (j, w) in enumerate(jmeta_shared[n]):
                if (j, w) in own_jobs:
                    dv[:, 2 * k] = own_jobs[(j, w)][0]
                    dv[:, 2 * k + 1] = own_jobs[(j, w)][1]
                else:
                    dv[:, 2 * k] = -1.0
            m[f"idx_{n}"] = idxp
            m[f"dv_{n}"] = dv
        # tables
        m["x_pad16"] = x_pad16
        m["xh_pad16"] = xh_pad16
        m["T12"] = T12
        m["T34"] = T34
        m["T5"] = T5
        m["T_h"] = T_h
        # consts
        m["iota"] = np.tile(np.arange(P, dtype=f32), (P, 1))
        m["identity"] = np.eye(P, dtype=f32)
        m["onesrow"] = np.ones((1, P), f32)
        m["Wx_enc"] = Wx_enc
        m["W12T"] = W12T
        m["W34T"] = W34T
        m["W5T"] = W5T
        m["bp_row"] = bp.reshape(1, H)
        for l in range(2):
            pl = per_layer[l]
            m[f"WlT_tm_{l}"] = pl["WlT_tm"]
            m[f"WlT_en_{l}"] = pl["WlT_en"]
            m[f"Ch_x_{l}"] = pl["Ch_x"]
            m[f"Ch_t_{l}"] = pl["Ch_t"]
            m[f"WrT_{l}"] = pl["WrT"]
            m[f"bias_{l}"] = pl["bias"].reshape(1, H)
        # pooling indicators [nwin, 128, 16]
        ind = np.zeros((nwin, P, 16), f32)
        gbase = np.zeros(nwin, np.int32)
        for w in range(nwin):
            r0 = w * P
            rows = np.arange(r0, min(r0 + P, PC))
            g0 = r0 // 10
            gbase[w] = g0
            for r in rows:
                g = r // 10
                if g - g0 < 16:
                    ind[w, r - r0, g - g0] = 0.1
        m["poolind"] = ind
        m["WcT"] = Wc.T.copy()          # [128, 1]
        in_maps.append(m)

    cfg = dict(shape_cfg=shape_cfg, jmeta=jmeta_shared, nwin=nwin,
               gbase=gbase.tolist(), bc=float(bc[0]))
    return in_maps, cfg


# ------------------------------------------------------------ device build
def _build(cfg):
    nwin = cfg["nwin"]
    shape_cfg = cfg["shape_cfg"]
    jmeta = cfg["jmeta"]
    gbase = cfg["gbase"]

    nc = bacc.Bacc("TRN2", target_bir_lowering=False, debug=False,
                   num_devices=NC, dynamic_dma_scratch_size=65536)

    TBL = {
        "enc_x": ("x_pad16", [N_PLAYER, 16]),
        "enc_12": ("T12", [40000, 32]),
        "enc_34": ("T34", [40000, 32]),
        "enc_5": ("T5", [200, 16]),
        "h_x": ("xh_pad16", [N_HIST, 16]),
        "h_t": ("T_h", [20000, 32]),
    }
    dram_in = {}
    for name, shp in [("x_pad16", [N_PLAYER, 16]), ("xh_pad16", [N_HIST, 16]),
                      ("T12", [40000, 32]), ("T34", [40000, 32]),
                      ("T5", [200, 16]), ("T_h", [20000, 32]),
                      ("iota", [P, P]), ("identity", [P, P]),
                      ("onesrow", [1, P]), ("Wx_enc", [16, H]),
                      ("W12T", [32, H]), ("W34T", [32, H]), ("W5T", [16, H]),
                      ("bp_row", [1, H]), ("poolind", [nwin, P, 16]),
                      ("WcT", [H, 1])]:
        dram_in[name] = nc.dram_tensor(name, shp, F32, kind="ExternalInput")
    for l in range(2):
        for name, shp in [(f"WlT_tm_{l}", [H, H]), (f"WlT_en_{l}", [H, H]),
                          (f"Ch_x_{l}", [16, H]), (f"Ch_t_{l}", [32, H]),
                          (f"WrT_{l}", [H, H]), (f"bias_{l}", [1, H])]:
            dram_in[name] = nc.dram_tensor(name, shp, F32, kind="ExternalInput")
    for n, (nchunk, njob) in shape_cfg.items():
        dram_in[f"idx_{n}"] = nc.dram_tensor(f"idx_{n}", [P, nchunk], I32,
                                             kind="ExternalInput")
        dram_in[f"dv_{n}"] = nc.dram_tensor(f"dv_{n}", [P, 2 * njob], F32,
                                            kind="ExternalInput")
    y_out = nc.dram_tensor("y", [1, GC], F32, kind="ExternalOutput")

    with tile.TileContext(nc) as tc, \
         tc.tile_pool(name="const", bufs=1) as constp, \
         tc.tile_pool(name="meta", bufs=1) as metap, \
         tc.tile_pool(name="v", bufs=4) as vp, \
         tc.tile_pool(name="oh", bufs=4) as ohp, \
         tc.tile_pool(name="st", bufs=4) as stp, \
         tc.tile_pool(name="hist", bufs=1) as histp, \
         tc.tile_pool(name="out", bufs=4) as outp, \
         tc.tile_pool(name="yrowp", bufs=1) as yrowp, \
         tc.tile_pool(name="psumA", bufs=2, space="PSUM") as psp, \
         tc.tile_pool(name="psumB", bufs=1, space="PSUM") as pspB, \
         tc.tile_pool(name="dram", bufs=1, space="DRAM") as dramp:

        # ---- constants to SBUF
        C = {}
        for name in ["iota", "identity", "onesrow", "Wx_enc", "W12T", "W34T",
                     "W5T", "bp_row", "WcT"] + \
                [f"{w}_{l}" for l in range(2)
                 for w in ["WlT_tm", "WlT_en", "Ch_x", "Ch_t", "WrT", "bias"]]:
            shp = dram_in[name].shape
            t = constp.tile(list(shp), F32, tag=f"c_{name}")
            nc.sync.dma_start(t[:], dram_in[name][:])
            C[name] = t

        # ---- stream metadata to SBUF
        SM = {}
        for n, (nchunk, njob) in shape_cfg.items():
            it = metap.tile([P, nchunk], I32, tag=f"ix_{n}")
            nc.sync.dma_start(it[:], dram_in[f"idx_{n}"][:])
            dt_ = metap.tile([P, 2 * njob], F32, tag=f"dv_{n}")
            nc.sync.dma_start(dt_[:], dram_in[f"dv_{n}"][:])
            SM[n] = (it, dt_)

        # ---- DRAM intermediates
        p0_own = dramp.tile([PC, H], F32)
        p1_own = dramp.tile([PC, H], F32)
        p0_full = dramp.tile([N_PLAYER, H], F32)
        p1_full = dramp.tile([N_PLAYER, H], F32)
        pT0 = dramp.tile([P, PC], F32)
        pT1 = dramp.tile([P, PC], F32)

        # hist persistent sT slabs (DRAM) + zero tile
        sTh_x_d = dramp.tile([16, nwin * P], F32)
        sTh_t_d = dramp.tile([32, nwin * P], F32)
        pooledT = histp.tile([P, GC + 32], F32)
        nc.gpsimd.memset(pooledT[:], 0.0)
        zt32 = histp.tile([32, P], F32)
        nc.gpsimd.memset(zt32[:], 0.0)

        def jobs_by_window(sname):
            """window -> list of (job_k, chunk_j)"""
            by_w = {}
            for k, (j, w) in enumerate(jmeta[sname]):
                by_w.setdefault(w, []).append((k, j))
            return by_w

        def run_stream(sname, table_ap, width, w, memo):
            """Run all jobs of stream sname for window w.
            Returns psum tile [width, 128] (sT) or None."""
            by_w = jobs_by_window(sname)
            jobs = by_w.get(w, [])
            if not jobs:
                return None
            it, dt_ = SM[sname]
            ps = psp.tile([width, P], F32, tag="ps")
            for k_i, (k, j) in enumerate(jobs):
                if j in memo:
                    V = memo[j]
                else:
                    V = vp.tile([P, width], F32, tag=f"V_{sname}")
                    nc.gpsimd.indirect_dma_start(
                        out=V[:, :], out_offset=None, in_=table_ap,
                        in_offset=bass.IndirectOffsetOnAxis(
                            ap=it[:, j:j + 1], axis=0),
                    )
                    memo.clear()
                    memo[j] = V
                oh = ohp.tile([P, P], F32, tag=f"oh_{sname}")
                nc.vector.tensor_tensor(
                    out=oh[:], in0=C["iota"][:],
                    in1=dt_[:, 2 * k:2 * k + 1].to_broadcast([P, P]),
                    op=mybir.AluOpType.is_equal)
                nc.vector.tensor_scalar_mul(
                    oh[:], oh[:], dt_[:, 2 * k + 1:2 * k + 2])
                nc.tensor.matmul(ps[:], lhsT=V[:, :], rhs=oh[:],
                                 start=(k_i == 0), stop=(k_i == len(jobs) - 1))
            sb = stp.tile([width, P], F32, tag=f"s_{sname}")
            nc.scalar.copy(sb[:], ps[:])
            return sb

        def combine(w, terms, relu, bias_row):
            """terms: list of (lhsT_ap [K,128-ish], rhs_ap [K, H]).
            Returns sbuf tile [128, H] (rows beyond window size = garbage)."""
            po = psp.tile([P, H], F32, tag="po")
            for i, (lh, rh) in enumerate(terms):
                m = lh.shape[-1]
                nc.tensor.matmul(po[:m, :], lhsT=lh, rhs=rh, start=(i == 0),
                                 stop=False)
            nc.tensor.matmul(po[:], lhsT=C["onesrow"][:], rhs=bias_row,
                             start=False, stop=True)
            ot = outp.tile([P, H], F32, tag="ot")
            if relu:
                nc.scalar.activation(ot[:], po[:],
                                     mybir.ActivationFunctionType.Relu)
            else:
                nc.scalar.copy(ot[:], po[:])
            return ot

        def transpose_to(ot, w, wsize, slab):
            pt = pspB.tile([P, P], F32, tag="pt")
            nc.tensor.transpose(out=pt[:], in_=ot[:], identity=C["identity"][:])
            ts = outp.tile([P, P], F32, tag="ts")
            nc.scalar.copy(ts[:], pt[:])
            nc.sync.dma_start(slab[:, w * P:w * P + wsize], ts[:, :wsize])

        # ================= ENC phase =================
        memo = {n: {} for n in shape_cfg}
        for w in range(nwin):
            wsize = min(P, PC - w * P)
            t_x = run_stream("enc_x", dram_in["x_pad16"][:, :], 16, w,
                             memo["enc_x"])
            t_12 = run_stream("enc_12", dram_in["T12"][:, :], 32, w,
                              memo["enc_12"])
            t_34 = run_stream("enc_34", dram_in["T34"][:, :], 32, w,
                              memo["enc_34"])
            t_5 = run_stream("enc_5", dram_in["T5"][:, :], 16, w,
                             memo["enc_5"])
            terms = []
            for sb, wt in [(t_x, "Wx_enc"), (t_12, "W12T"),
                           (t_34, "W34T"), (t_5, "W5T")]:
                if sb is not None:
                    terms.append((sb[:], C[wt][:]))
            ot = combine(w, terms, relu=False, bias_row=C["bp_row"][:])
            nc.sync.dma_start(p0_own[w * P:w * P + wsize, :], ot[:wsize, :])
            transpose_to(ot, w, wsize, pT0)

        # ================= HIST phase =================
        for w in range(nwin):
            t_hx = run_stream("h_x", dram_in["xh_pad16"][:, :], 16, w,
                              memo["h_x"])
            t_ht = run_stream("h_t", dram_in["T_h"][:, :], 32, w, memo["h_t"])
            nc.sync.dma_start(sTh_x_d[:, w * P:(w + 1) * P],
                              t_hx[:] if t_hx is not None else zt32[:16, :])
            nc.sync.dma_start(sTh_t_d[:, w * P:(w + 1) * P],
                              t_ht[:] if t_ht is not None else zt32[:, :])

        # ================= AllGather p0 =================
        nc.gpsimd.collective_compute(
            "AllGather", mybir.AluOpType.bypass,
            replica_groups=[list(range(NC))],
            ins=[p0_own.opt()], outs=[p0_full.opt()])

        # ================= layers =================
        for l in range(2):
            memo["tm"].clear()
            memo["en"].clear()
            p_full = p0_full if l == 0 else p1_full
            pT_prev = pT0 if l == 0 else pT1
            p_own_next = p1_own
            for w in range(nwin):
                wsize = min(P, PC - w * P)
                t_tm = run_stream("tm", p_full[:, :], H, w, memo["tm"])
                t_en = run_stream("en", p_full[:, :], H, w, memo["en"])
                prev_t = outp.tile([P, P], F32, tag="prevT")
                nc.sync.dma_start(prev_t[:, :wsize],
                                  pT_prev[:, w * P:w * P + wsize])
                terms = []
                if t_tm is not None:
                    terms.append((t_tm[:], C[f"WlT_tm_{l}"][:]))
                if t_en is not None:
                    terms.append((t_en[:], C[f"WlT_en_{l}"][:]))
                shx = stp.tile([16, P], F32, tag="shx")
                nc.sync.dma_start(shx[:], sTh_x_d[:, w * P:(w + 1) * P])
                sht = stp.tile([32, P], F32, tag="sht")
                nc.sync.dma_start(sht[:], sTh_t_d[:, w * P:(w + 1) * P])
                terms.append((shx[:], C[f"Ch_x_{l}"][:]))
                terms.append((sht[:], C[f"Ch_t_{l}"][:]))
                terms.append((prev_t[:, :wsize], C[f"WrT_{l}"][:]))
                ot = combine(w, terms, relu=True, bias_row=C[f"bias_{l}"][:])
                if l == 0:
                    nc.sync.dma_start(p_own_next[w * P:w * P + wsize, :],
                                      ot[:wsize, :])
                    transpose_to(ot, w, wsize, pT1)
                else:
                    # pooling: psum_pool[f, 16] = ot.T @ ind_w
                    ind_t = outp.tile([P, 16], F32, tag="ind")
                    nc.sync.dma_start(ind_t[:], dram_in["poolind"][w])
                    pp = pspB.tile([P, 16], F32, tag="pp")
                    nc.tensor.matmul(pp[:], lhsT=ot[:], rhs=ind_t[:],
                                     start=True, stop=True)
                    g0 = gbase[w]
                    nc.vector.tensor_add(pooledT[:, g0:g0 + 16],
                                         pooledT[:, g0:g0 + 16],
                                         pp[:, :16])
            if l == 0:
                nc.gpsimd.collective_compute(
                    "AllGather", mybir.AluOpType.bypass,
                    replica_groups=[list(range(NC))],
                    ins=[p1_own.opt()], outs=[p1_full.opt()])
            if l == 1:
                pass

        # ================= output =================
        yrow = yrowp.tile([1, GC], F32, tag="yrow")
        for k0 in range(0, GC, 512):
            kn = min(512, GC - k0)
            po = pspB.tile([1, 512], F32, tag="yps")
            nc.tensor.matmul(po[:, :kn], lhsT=C["WcT"][:],
                             rhs=pooledT[:, k0:k0 + kn], start=True, stop=True)
            nc.scalar.add(yrow[:, k0:k0 + kn], po[:, :kn], cfg["bc"])
        nc.sync.dma_start(y_out[:, :], yrow[:])

    nc.compile()
    return nc


def kernel(**inputs):
    in_maps, cfg = _prep(inputs)
    nc = _build(cfg)
    trace = bool(os.environ.get("GNN_TRACE"))
    res = run_bass_kernel_spmd(nc, in_maps, core_ids=list(range(NC)),
                               trace=trace)
    LAST_EXEC_NS[0] = res.exec_time_ns
    out = np.concatenate([res.results[c]["y"].reshape(GC, 1)
                          for c in range(NC)], axis=0)
    return out.astype(np.float32)

